# revision 25
# baseline (speedup 1.0000x reference)
"""Distributed Trainium2 kernel for nn_Attention_68719477187.

RoPE + causal GQA attention (B=2, S=2048, DIM=2048, 32 q heads / 8 kv heads,
head_dim 64) on 8 NeuronCores: TP=8 over heads.

Per core c: 4 q heads {4c..4c+3} (2 pair-tiles rt) + 1 kv head (c), BOTH
batches.  Output is token-sharded: after attention, one 8-core AllToAll per
sequence sub-chunk redistributes attention outputs so core r owns
(batch r//4, a token slice), then each core runs the full wo on its tokens.
AllToAll moves 4x fewer bytes than an AllGather scheme (0.5MB/round vs 2MB).

  1. qkv: 3 row-tiles per (batch, chunk): 2x q-pairs + packed [k;v] tile.
     RoPE applied in transposed layout (64-dim pre-permuted evens|odds).
     v rows leave the packed tile via DMA-transpose into token-major vaug
     (ones column appended so the softmax denominator falls out of the AV
     matmul).
  2. scores.T tiles (keys on partitions) -> exp (no max subtraction; scores
     are O(5) so fp32 exp is safe) -> causal 0/1 mask on the diagonal 128
     cols -> AV matmul against vaug.
  3. Per sub-chunk: atile (attn.T, bf16) staged to DRAM, 8-core AllToAll,
     gather to [128p, 16kd, tok] and token-major wo: lhsT = attn.T block
     (stationary), rhs = wo.T slice (moving, 512 wide), PSUM accum over kd.
     Subs 3+4 share one wo chain (their gathers land in one tile) so the
     tail after the last attention is just a 128KB AllToAll + one wo chain.

Compute in bf16 (fp32 PSUM accumulation), output fp32.
"""

import sys

if "/opt/trn_rl_repo" not in sys.path:
    sys.path.insert(0, "/opt/trn_rl_repo")

import numpy as np
import ml_dtypes

from concourse import bacc, tile, mybir
from concourse.bass_utils import run_bass_kernel_spmd

BF16 = ml_dtypes.bfloat16

S = 2048          # sequence length
D = 2048          # model dim
HD = 64           # head dim
NB = 2            # batches (every core sees both)
QC = 512          # q chunk (matmul free dim)
NSC = S // QC     # 4 seq chunks
NKD = D // 128    # 16 contraction tiles
NKT = S // 128    # 16 key tiles
SCALE = HD ** -0.5

# attention/AllToAll/wo sub-chunks: (qT chunk, col offset, width).  The last
# 512 chunk is split 384+128 so the final serial a2a -> gather -> wo chain
# carries only 128KB; subs 3+4 share one wo chain (128 tokens/core).
SUBS = [(0, 0, QC), (1, 0, QC), (2, 0, QC), (3, 0, 384), (3, 384, 128)]
SUB_T4 = [qn // 4 for _, _, qn in SUBS]           # tokens/core per sub
SUB_BASE = [qc * QC + q0 for qc, q0, _ in SUBS]   # global token base
SUB_LOCAL = [0, 128, 256, 384, 480]               # local out row base

_NC = None


def _build():
    nc = bacc.Bacc("TRN2", target_bir_lowering=False, debug=False, num_devices=8)
    BF = mybir.dt.bfloat16
    F32 = mybir.dt.float32
    EXP = mybir.ActivationFunctionType.Exp

    # host-staged, per-partition-contiguous layouts (low descriptor counts)
    xS = nc.declare_dram_parameter("xS", [NB, NSC, 128, NKD, QC], BF, isOutput=False)
    wqS = nc.declare_dram_parameter("wqS", [128, NKD, 256], BF, isOutput=False)
    wkvS = nc.declare_dram_parameter("wkvS", [128, NKD, 128], BF, isOutput=False)
    woTS = nc.declare_dram_parameter("woTS", [128, NKD, D], BF, isOutput=False)
    cosS = nc.declare_dram_parameter("cosS", [128, S], F32, isOutput=False)
    sinS = nc.declare_dram_parameter("sinS", [128, S], F32, isOutput=False)
    # mask-fold constants: plane 0 = -BIG*I, plane 1 = strict-lower (q < d).
    # DIAG(-BIG) @ LOW = -BIG*(q < k): added into the diagonal score block in
    # PSUM so exp() underflows to 0 -- no separate DVE mask multiply.
    mmc = nc.declare_dram_parameter("mmc", [128, 2, 128], BF, isOutput=False)
    out = nc.declare_dram_parameter("out", [512, D], F32, isOutput=True)

    with tile.TileContext(nc) as tc:
        with (
            tc.tile_pool(name="wpool", bufs=1) as wpool,
            tc.tile_pool(name="pers", bufs=1) as pers,
            tc.tile_pool(name="dram", bufs=1, space="DRAM") as dram,
            tc.tile_pool(name="xpool", bufs=10) as xpool,
            tc.tile_pool(name="cspool", bufs=2) as cspool,
            tc.tile_pool(name="rtmp", bufs=2) as rtmp,
            tc.tile_pool(name="vtp", bufs=4) as vtp,
            tc.tile_pool(name="ppool", bufs=2) as ppool,
            tc.tile_pool(name="npool", bufs=2) as npool,
            tc.tile_pool(name="apool", bufs=4) as apool,
            tc.tile_pool(name="agp", bufs=2) as agp,
            tc.tile_pool(name="opool", bufs=2) as opool,
            tc.tile_pool(name="gps", bufs=2, space="PSUM") as gps,
            tc.tile_pool(name="stps", bufs=2, space="PSUM") as stps,
            tc.tile_pool(name="avps", bufs=2, space="PSUM") as avps,
        ):
            # ---- persistent weights / constants ----
            wq_sb = [wpool.tile([128, NKD // 4, 256], BF, name=f"wq_sb{h}",
                                tag=f"wq_sb{h}") for h in range(4)]
            wkv_sb = wpool.tile([128, NKD, 128], BF, name="wkv_sb", tag="wkv_sb")
            woT_sb = wpool.tile([128, NKD, D], BF, name="woT_sb", tag="woT_sb")
            mmc_sb = wpool.tile([128, 2, 128], BF, name="mmc_sb", tag="mmc_sb")

            # ---- persistent activations ----
            qT = [[[pers.tile([128, QC], BF, name=f"qT_{rt}_{b}_{sc}",
                              tag=f"qT_{rt}_{b}_{sc}") for sc in range(NSC)]
                   for b in range(NB)] for rt in range(2)]
            kdup = [[pers.tile([128, QC], BF, name=f"kd_{b}_{sc}",
                               tag=f"kd_{b}_{sc}") for sc in range(NSC)]
                    for b in range(NB)]
            vaug = [[pers.tile([128, 65], BF, name=f"va_{b}_{kt}",
                               tag=f"va_{b}_{kt}") for kt in range(NKT)]
                    for b in range(NB)]
            for b in range(NB):
                for kt in range(NKT):
                    nc.gpsimd.memset(vaug[b][kt][:, 64:65], 1.0)

            # AllToAll buffers: [8 dest/src ranks, 2rt, 128p, T4 tokens]
            # (rt before p so the gather's (src, rt) dims merge into one run)
            a2a_in = [dram.tile([8, 2, 128, SUB_T4[i]], BF, name=f"a2a_in_{i}")
                      for i in range(len(SUBS))]
            a2a_out = [dram.tile([8, 2, 128, SUB_T4[i]], BF, name=f"a2a_out_{i}")
                       for i in range(len(SUBS))]

            # ---- hoisted loads.  SP queue: loads + v-transposes + gathers;
            # ACT queue: a2a staging + out writes (never wait on collectives
            # before compute-dependent DMAs in FIFO order).
            xts = {}
            cs_tiles = {}

            def load_x(b, sc):
                parts = []
                for h in range(4):
                    xt = xpool.tile([128, NKD // 4, QC], BF, name="xt", tag="xt")
                    nc.sync.dma_start(xt[:], xS[b, sc, :, h * 4:(h + 1) * 4, :])
                    parts.append(xt)
                xts[(b, sc)] = parts

            def load_cs(sc):
                ct = cspool.tile([128, QC], F32, name="cosc", tag="cosc")
                st = cspool.tile([128, QC], F32, name="sinc", tag="sinc")
                nc.sync.dma_start(ct[:], cosS[:, sc * QC:(sc + 1) * QC])
                nc.sync.dma_start(st[:], sinS[:, sc * QC:(sc + 1) * QC])
                cs_tiles[sc] = (ct, st)

            def load_wo(h):
                nc.sync.dma_start(woT_sb[:, h * 4:(h + 1) * 4, :],
                                  woTS[:, h * 4:(h + 1) * 4, :])

            # interleave wq / x(b0,0) quarters so the first matmul starts
            # after ~1MB of DMA
            nc.sync.dma_start(wq_sb[0][:], wqS[:, 0:4, :])
            xts[(0, 0)] = []
            for h in range(4):
                xt = xpool.tile([128, NKD // 4, QC], BF, name="xt", tag="xt")
                nc.sync.dma_start(xt[:], xS[0, 0, :, h * 4:(h + 1) * 4, :])
                xts[(0, 0)].append(xt)
                if h < 3:
                    nc.sync.dma_start(wq_sb[h + 1][:],
                                      wqS[:, (h + 1) * 4:(h + 2) * 4, :])
            load_cs(0)
            nc.sync.dma_start(wkv_sb[:], wkvS[:])
            nc.sync.dma_start(mmc_sb[:], mmc[:])
            load_x(1, 0)
            load_x(0, 1)

            PENDING = [
                lambda: load_x(1, 1),
                lambda: load_cs(1),
                lambda: load_x(0, 2),
                lambda: load_wo(0),
                lambda: load_x(1, 2),
                lambda: load_cs(2),
                lambda: load_x(0, 3),
                lambda: load_wo(1),
                lambda: load_x(1, 3),
                lambda: load_cs(3),
                lambda: load_wo(2),
                lambda: load_wo(3),
            ]

            def consume_pending(k):
                for _ in range(k):
                    if PENDING:
                        PENDING.pop(0)()

            def rope(ps, sc, dst, nr):
                """nr-row rope: dst = raw*cos + swap32(raw)*sin_signed."""
                ct, sn = cs_tiles[sc]
                raw = rtmp.tile([128, QC], F32, name="raw", tag="raw")
                nc.scalar.copy(raw[0:nr, :], ps[0:nr, :])
                rot = rtmp.tile([128, QC], F32, name="rot", tag="rot")
                for b32 in range(nr // 32):
                    src = (b32 ^ 1) * 32
                    nc.gpsimd.tensor_copy(rot[b32 * 32:(b32 + 1) * 32, :],
                                          raw[src:src + 32, :])
                t1 = rtmp.tile([128, QC], F32, name="t1", tag="t1")
                nc.vector.tensor_mul(t1[0:nr, :], raw[0:nr, :], ct[0:nr, :])
                nc.vector.tensor_mul(rot[0:nr, :], rot[0:nr, :], sn[0:nr, :])
                nc.vector.tensor_add(dst[0:nr, :], t1[0:nr, :], rot[0:nr, :])

            def qkv_block(b, sc):
                xt = xts[(b, sc)]
                for rt in range(2):
                    ps = gps.tile([128, QC], F32, name="gp", tag="gp")
                    for kd in range(NKD):
                        nc.tensor.matmul(
                            ps[:], wq_sb[kd // 4][:, kd % 4, rt * 128:(rt + 1) * 128],
                            xt[kd // 4][:, kd % 4, :],
                            start=(kd == 0), stop=(kd == NKD - 1))
                    rope(ps, sc, qT[rt][b][sc], 128)
                # packed [k(64, perm'd); v(64, natural)] tile
                ps = gps.tile([128, QC], F32, name="gp", tag="gp")
                for kd in range(NKD):
                    nc.tensor.matmul(ps[:], wkv_sb[:, kd, :],
                                     xt[kd // 4][:, kd % 4, :],
                                     start=(kd == 0), stop=(kd == NKD - 1))
                # k rows 0:64 -> rope -> duplicate into both kdup halves
                kr = rtmp.tile([64, QC], BF, name="kr", tag="kr")
                rope(ps, sc, kr, 64)
                nc.gpsimd.tensor_copy(kdup[b][sc][0:64, :], kr[:])
                nc.gpsimd.tensor_copy(kdup[b][sc][64:128, :], kr[:])
                # v rows 64:128 -> bf16 -> DMA-transpose into token-major vaug
                vtmp = vtp.tile([64, QC], BF, name="vtmp", tag="vtmp")
                nc.scalar.copy(vtmp[:], ps[64:128, :])
                for tt in range(4):
                    kt = sc * 4 + tt
                    nc.scalar.dma_start_transpose(
                        vaug[b][kt][:, 0:64], vtmp[:, tt * 128:(tt + 1) * 128])

            def attn_phase(i):
                qc, q0, qn = SUBS[i]
                gqs = qc * QC + q0          # 128-aligned global q start
                t0 = gqs // 128             # first diagonal key tile
                nkt = (gqs + qn) // 128     # causal: key tiles up to sub end
                for b in range(NB):
                    atile = apool.tile([128, 2, QC], BF, name="atile", tag="atile")
                    for rt in range(2):
                        avs = [avps.tile([65, QC], F32, name="av", tag="av")
                               for _ in range(2)]
                        for kt in range(nkt):
                            kb = (kt % 4) * 128
                            m = kt - t0
                            qo = 128 * m if m > 0 else 0
                            n = qn - qo
                            st = stps.tile([128, 2, QC], F32, name="st", tag="st")
                            for half in range(2):
                                lo, hi = half * 64, half * 64 + 64
                                nc.tensor.matmul(
                                    st[:, half, 0:n],
                                    kdup[b][kt // 4][lo:hi, kb:kb + 128],
                                    qT[rt][b][qc][lo:hi, q0 + qo:q0 + qn],
                                    start=True, stop=(m < 0))
                                if m >= 0:  # fold causal mask into the PSUM
                                    nc.tensor.matmul(
                                        st[:, half, 0:128], mmc_sb[:, 0, :],
                                        mmc_sb[:, 1, :], start=False, stop=True)
                            p = ppool.tile([128, 2, QC], BF, name="p", tag="p")
                            nc.scalar.activation(p[:, :, 0:n], st[:, :, 0:n],
                                                 EXP, scale=SCALE)
                            for half in range(2):
                                nc.tensor.matmul(avs[half][:, qo:qn],
                                                 vaug[b][kt][:, :],
                                                 p[:, half, 0:n],
                                                 start=(kt == 0),
                                                 stop=(kt == nkt - 1))
                        for half in range(2):
                            av = avs[half]
                            recip = npool.tile([1, QC], F32, name="recip",
                                               tag="recip")
                            nc.vector.reciprocal(recip[:, 0:qn], av[64:65, 0:qn])
                            rb = npool.tile([64, QC], F32, name="rb", tag="rb")
                            nc.gpsimd.partition_broadcast(rb[:, 0:qn],
                                                          recip[:, 0:qn])
                            nc.vector.tensor_mul(
                                atile[half * 64:(half + 1) * 64, rt, 0:qn],
                                av[0:64, 0:qn], rb[:, 0:qn])
                    # stage to DRAM (ACT queue): dest 4b+d gets token block d
                    for r in range(2):
                        nc.scalar.dma_start(
                            a2a_in[i][4 * b:4 * b + 4, r, :, :]
                            .rearrange("d p t -> p d t"),
                            atile[:, r, 0:qn].rearrange("p (d t) -> p d t",
                                                        d=4))
                nc.gpsimd.collective_compute(
                    "AllToAll", mybir.AluOpType.bypass,
                    replica_groups=[[0, 1, 2, 3, 4, 5, 6, 7]],
                    ins=[a2a_in[i].opt()], outs=[a2a_out[i].opt()])

            agts = {}

            def gather(i, agt=None, col0=0):
                t4 = SUB_T4[i]
                if agt is None:
                    agt = agp.tile([128, NKD, 128], BF, name="agt", tag="agt")
                nc.sync.dma_start(
                    agt[:, :, col0:col0 + t4],
                    a2a_out[i].rearrange("c r p t -> p (c r) t"))
                agts[i] = agt
                return agt

            def wo_chain(i, ntok):
                """token-major wo for sub i's gathered tokens (<=128)."""
                agt = agts[i]
                for oh in range(2):
                    ot = opool.tile([128, D // 2], F32, name="ot", tag="ot")
                    for oc in range(2):
                        ps = gps.tile([128, QC], F32, name="gp", tag="gp")
                        for kd in range(NKD):
                            nc.tensor.matmul(
                                ps[0:ntok, :], agt[:, kd, 0:ntok],
                                woT_sb[:, kd, (2 * oh + oc) * QC:
                                       (2 * oh + oc + 1) * QC],
                                start=(kd == 0), stop=(kd == NKD - 1))
                        nc.vector.tensor_copy(ot[0:ntok, oc * QC:(oc + 1) * QC],
                                              ps[0:ntok, :])
                    nc.scalar.dma_start(
                        out[SUB_LOCAL[i]:SUB_LOCAL[i] + ntok,
                            oh * (D // 2):(oh + 1) * (D // 2)],
                        ot[0:ntok, :])

            # ---- schedule ----
            # i=0: qkv(0) attn(0) | i=1: qkv(1) attn(1) | i=2: qkv(2) attn(2)
            # wo(0) | i=3: qkv(3) attn(3) wo(1) wo(2) | i=4: attn(4) wo(3+4)
            for i, (qc, q0, qn) in enumerate(SUBS):
                if q0 == 0:
                    qkv_block(0, qc)
                    consume_pending(1)
                    qkv_block(1, qc)
                    consume_pending(3)
                attn_phase(i)
                if i == 2:
                    gather(0)
                    wo_chain(0, 128)
                if i == 3:
                    gather(1)
                    wo_chain(1, 128)
                    gather(2)
                    wo_chain(2, 128)
            # tail: wo(3) fills the a2a(4) window (keeps the PE clock warm),
            # wo(4) is the only post-collective work
            gather(3)
            wo_chain(3, 96)
            gather(4)
            wo_chain(4, 32)

    nc.compile()
    return nc


def _get_nc():
    global _NC
    if _NC is None:
        _NC = _build()
    return _NC


def _prepare_in_maps(x, freqs_cis, wqkv, wo):
    x = np.asarray(x)
    freqs_cis = np.asarray(freqs_cis)
    wqkv = np.asarray(wqkv)
    wo = np.asarray(wo)

    perm = np.concatenate([np.arange(0, HD, 2), np.arange(1, HD, 2)])
    cos = np.ascontiguousarray(freqs_cis[:, :, 0].T)  # (32, S)
    sin = np.ascontiguousarray(freqs_cis[:, :, 1].T)
    cosS = np.ascontiguousarray(np.concatenate([cos, cos, cos, cos], axis=0),
                                dtype=np.float32)
    sinS = np.ascontiguousarray(np.concatenate([-sin, sin, -sin, sin], axis=0),
                                dtype=np.float32)
    p_i = np.arange(128)[:, None]
    f_i = np.arange(128)[None, :]
    mmcA = np.stack([-30000.0 * (f_i == p_i), 1.0 * (f_i < p_i)],
                    axis=1).astype(BF16)

    def stage(wt):
        # (D, C) with D = 16*128 -> (128, 16, C), per-partition contiguous
        return np.ascontiguousarray(
            wt.reshape(NKD, 128, wt.shape[1]).transpose(1, 0, 2)).astype(BF16)

    xSs = np.empty((NB, NSC, 128, NKD, QC), dtype=BF16)
    for b in range(NB):
        xt = x[b].T  # (D, S)
        xSs[b] = xt.reshape(NKD, 128, NSC, QC).transpose(2, 1, 0, 3)
    xSs = np.ascontiguousarray(xSs)

    # wo.T staged by attention-column order: kd=(src_core, rt), p=half*64+d
    # -> attn col (4*src + 2*rt + half)*64 + d ; identical for every core.
    p_idx = np.arange(128)
    kd_idx = np.arange(NKD)
    cols = ((4 * (kd_idx[None, :] // 2) + 2 * (kd_idx[None, :] % 2)
             + (p_idx[:, None] // 64)) * 64 + (p_idx[:, None] % 64))
    woTS = np.ascontiguousarray(
        wo[:, cols].transpose(1, 2, 0)).astype(BF16)  # [128, 16, 2048]

    in_maps = []
    for c in range(8):
        qrows = np.concatenate([(4 * c + h) * HD + perm for h in range(4)])
        krows = D + c * HD + perm
        vrows = D + 512 + c * HD + np.arange(HD)
        kvrows = np.concatenate([krows, vrows])
        in_maps.append({
            "xS": xSs,
            "wqS": stage(wqkv[qrows, :].T),
            "wkvS": stage(wqkv[kvrows, :].T),
            "woTS": woTS,
            "cosS": cosS,
            "sinS": sinS,
            "mmc": mmcA,
        })
    return in_maps


def kernel(x, freqs_cis, wqkv, wo, _trace=False):
    in_maps = _prepare_in_maps(x, freqs_cis, wqkv, wo)
    res = run_bass_kernel_spmd(_get_nc(), in_maps, core_ids=list(range(8)),
                               trace=_trace)

    outf = np.empty((2, S, D), np.float32)
    for c in range(8):
        b, blk = c // 4, c % 4
        o = res.results[c]["out"]  # [512, 2048] fp32
        for i in range(len(SUBS)):
            t4 = SUB_T4[i]
            g0 = SUB_BASE[i] + blk * t4
            l0 = SUB_LOCAL[i]
            outf[b, g0:g0 + t4, :] = o[l0:l0 + t4, :]
    if _trace:
        kernel.last_exec_time_ns = res.exec_time_ns
        kernel.last_results = res
    return outf


# revision 28
# speedup vs baseline: 1.0057x; 1.0057x over previous
"""Distributed Trainium2 kernel for nn_Attention_68719477187.

RoPE + causal GQA attention (B=2, S=2048, DIM=2048, 32 q heads / 8 kv heads,
head_dim 64) on 8 NeuronCores: TP=8 over heads.

Per core c: 4 q heads {4c..4c+3} (2 pair-tiles rt) + 1 kv head (c), BOTH
batches.  Output is token-sharded: after attention, one 8-core AllToAll per
sequence sub-chunk redistributes attention outputs so core r owns
(batch r//4, a token slice), then each core runs the full wo on its tokens.
AllToAll moves 4x fewer bytes than an AllGather scheme (0.5MB/round vs 2MB).

  1. qkv: 3 row-tiles per (batch, chunk): 2x q-pairs + packed [k;v] tile.
     RoPE applied in transposed layout (64-dim pre-permuted evens|odds).
     v rows leave the packed tile via DMA-transpose into token-major vaug
     (ones column appended so the softmax denominator falls out of the AV
     matmul).
  2. scores.T tiles (keys on partitions) -> exp (no max subtraction; scores
     are O(5) so fp32 exp is safe) -> causal 0/1 mask on the diagonal 128
     cols -> AV matmul against vaug.
  3. Per sub-chunk: atile (attn.T, bf16) staged to DRAM, 8-core AllToAll,
     gather to [128p, 16kd, tok] and token-major wo: lhsT = attn.T block
     (stationary), rhs = wo.T slice (moving, 512 wide), PSUM accum over kd.
     Subs 3+4 share one wo chain (their gathers land in one tile) so the
     tail after the last attention is just a 128KB AllToAll + one wo chain.

Compute in bf16 (fp32 PSUM accumulation), output fp32.
"""

import sys

if "/opt/trn_rl_repo" not in sys.path:
    sys.path.insert(0, "/opt/trn_rl_repo")

import numpy as np
import ml_dtypes

from concourse import bacc, tile, mybir
from concourse.bass_utils import run_bass_kernel_spmd

BF16 = ml_dtypes.bfloat16

S = 2048          # sequence length
D = 2048          # model dim
HD = 64           # head dim
NB = 2            # batches (every core sees both)
QC = 512          # q chunk (matmul free dim)
NSC = S // QC     # 4 seq chunks
NKD = D // 128    # 16 contraction tiles
NKT = S // 128    # 16 key tiles
SCALE = HD ** -0.5

# attention/AllToAll/wo sub-chunks: (qT chunk, col offset, width).  The last
# 512 chunk is split 384+128 so the final serial a2a -> gather -> wo chain
# carries only 128KB; subs 3+4 share one wo chain (128 tokens/core).
SUBS = [(0, 0, QC), (1, 0, QC), (2, 0, QC), (3, 0, 384), (3, 384, 128)]
SUB_T4 = [qn // 4 for _, _, qn in SUBS]           # tokens/core per sub
SUB_BASE = [qc * QC + q0 for qc, q0, _ in SUBS]   # global token base
SUB_LOCAL = [0, 128, 256, 384, 480]               # local out row base

_NC = None


def _build():
    nc = bacc.Bacc("TRN2", target_bir_lowering=False, debug=False, num_devices=8)
    BF = mybir.dt.bfloat16
    F32 = mybir.dt.float32
    EXP = mybir.ActivationFunctionType.Exp

    # host-staged, per-partition-contiguous layouts (low descriptor counts)
    xS = nc.declare_dram_parameter("xS", [NB, NSC, 128, NKD, QC], BF, isOutput=False)
    wqS = nc.declare_dram_parameter("wqS", [128, NKD, 256], BF, isOutput=False)
    wkvS = nc.declare_dram_parameter("wkvS", [128, NKD, 128], BF, isOutput=False)
    woTS = nc.declare_dram_parameter("woTS", [128, NKD, D], BF, isOutput=False)
    cosS = nc.declare_dram_parameter("cosS", [128, S], F32, isOutput=False)
    sinS = nc.declare_dram_parameter("sinS", [128, S], F32, isOutput=False)
    # mask-fold constants: plane 0 = -BIG*I, plane 1 = strict-lower (q < d).
    # DIAG(-BIG) @ LOW = -BIG*(q < k): added into the diagonal score block in
    # PSUM so exp() underflows to 0 -- no separate DVE mask multiply.
    mmc = nc.declare_dram_parameter("mmc", [128, 2, 128], BF, isOutput=False)
    out = nc.declare_dram_parameter("out", [512, D], F32, isOutput=True)

    with tile.TileContext(nc) as tc:
        with (
            tc.tile_pool(name="wpool", bufs=1) as wpool,
            tc.tile_pool(name="pers", bufs=1) as pers,
            tc.tile_pool(name="dram", bufs=1, space="DRAM") as dram,
            tc.tile_pool(name="xpool", bufs=10) as xpool,
            tc.tile_pool(name="cspool", bufs=2) as cspool,
            tc.tile_pool(name="rtmp", bufs=2) as rtmp,
            tc.tile_pool(name="vtp", bufs=4) as vtp,
            tc.tile_pool(name="ppool", bufs=3) as ppool,
            tc.tile_pool(name="npool", bufs=2) as npool,
            tc.tile_pool(name="apool", bufs=4) as apool,
            tc.tile_pool(name="agp", bufs=2) as agp,
            tc.tile_pool(name="opool", bufs=2) as opool,
            tc.tile_pool(name="gps", bufs=2, space="PSUM") as gps,
            tc.tile_pool(name="stps", bufs=2, space="PSUM") as stps,
            tc.tile_pool(name="avps", bufs=2, space="PSUM") as avps,
        ):
            # ---- persistent weights / constants ----
            wq_sb = [wpool.tile([128, NKD // 4, 256], BF, name=f"wq_sb{h}",
                                tag=f"wq_sb{h}") for h in range(4)]
            wkv_sb = wpool.tile([128, NKD, 128], BF, name="wkv_sb", tag="wkv_sb")
            woT_sb = wpool.tile([128, NKD, D], BF, name="woT_sb", tag="woT_sb")
            mmc_sb = wpool.tile([128, 2, 128], BF, name="mmc_sb", tag="mmc_sb")

            # ---- persistent activations ----
            qT = [[[pers.tile([128, QC], BF, name=f"qT_{rt}_{b}_{sc}",
                              tag=f"qT_{rt}_{b}_{sc}") for sc in range(NSC)]
                   for b in range(NB)] for rt in range(2)]
            kdup = [[pers.tile([128, QC], BF, name=f"kd_{b}_{sc}",
                               tag=f"kd_{b}_{sc}") for sc in range(NSC)]
                    for b in range(NB)]
            vaug = [[pers.tile([128, 65], BF, name=f"va_{b}_{kt}",
                               tag=f"va_{b}_{kt}") for kt in range(NKT)]
                    for b in range(NB)]
            for b in range(NB):
                for kt in range(NKT):
                    nc.gpsimd.memset(vaug[b][kt][:, 64:65], 1.0)

            # AllToAll buffers: [8 dest/src ranks, 2rt, 128p, T4 tokens]
            # (rt before p so the gather's (src, rt) dims merge into one run)
            a2a_in = [dram.tile([8, 2, 128, SUB_T4[i]], BF, name=f"a2a_in_{i}")
                      for i in range(len(SUBS))]
            a2a_out = [dram.tile([8, 2, 128, SUB_T4[i]], BF, name=f"a2a_out_{i}")
                       for i in range(len(SUBS))]

            # ---- hoisted loads.  SP queue: loads + v-transposes + gathers;
            # ACT queue: a2a staging + out writes (never wait on collectives
            # before compute-dependent DMAs in FIFO order).
            xts = {}
            cs_tiles = {}

            def load_x(b, sc):
                parts = []
                for h in range(4):
                    xt = xpool.tile([128, NKD // 4, QC], BF, name="xt", tag="xt")
                    nc.sync.dma_start(xt[:], xS[b, sc, :, h * 4:(h + 1) * 4, :])
                    parts.append(xt)
                xts[(b, sc)] = parts

            def load_cs(sc):
                ct = cspool.tile([128, QC], F32, name="cosc", tag="cosc")
                st = cspool.tile([128, QC], F32, name="sinc", tag="sinc")
                nc.sync.dma_start(ct[:], cosS[:, sc * QC:(sc + 1) * QC])
                nc.sync.dma_start(st[:], sinS[:, sc * QC:(sc + 1) * QC])
                cs_tiles[sc] = (ct, st)

            def load_wo(h):
                nc.sync.dma_start(woT_sb[:, h * 4:(h + 1) * 4, :],
                                  woTS[:, h * 4:(h + 1) * 4, :])

            # interleave wq / x(b0,0) quarters so the first matmul starts
            # after ~1MB of DMA
            nc.sync.dma_start(wq_sb[0][:], wqS[:, 0:4, :])
            xts[(0, 0)] = []
            for h in range(4):
                xt = xpool.tile([128, NKD // 4, QC], BF, name="xt", tag="xt")
                nc.sync.dma_start(xt[:], xS[0, 0, :, h * 4:(h + 1) * 4, :])
                xts[(0, 0)].append(xt)
                if h < 3:
                    nc.sync.dma_start(wq_sb[h + 1][:],
                                      wqS[:, (h + 1) * 4:(h + 2) * 4, :])
            load_cs(0)
            nc.sync.dma_start(wkv_sb[:], wkvS[:])
            nc.sync.dma_start(mmc_sb[:], mmc[:])
            load_x(1, 0)
            load_x(0, 1)

            PENDING = [
                lambda: load_x(1, 1),
                lambda: load_cs(1),
                lambda: load_x(0, 2),
                lambda: load_wo(0),
                lambda: load_x(1, 2),
                lambda: load_cs(2),
                lambda: load_x(0, 3),
                lambda: load_wo(1),
                lambda: load_x(1, 3),
                lambda: load_cs(3),
                lambda: load_wo(2),
                lambda: load_wo(3),
            ]

            def consume_pending(k):
                for _ in range(k):
                    if PENDING:
                        PENDING.pop(0)()

            def rope(ps, sc, dst, nr):
                """nr-row rope: dst = raw*cos + swap32(raw)*sin_signed."""
                ct, sn = cs_tiles[sc]
                raw = rtmp.tile([128, QC], F32, name="raw", tag="raw")
                nc.scalar.copy(raw[0:nr, :], ps[0:nr, :])
                rot = rtmp.tile([128, QC], F32, name="rot", tag="rot")
                for b32 in range(nr // 32):
                    src = (b32 ^ 1) * 32
                    nc.gpsimd.tensor_copy(rot[b32 * 32:(b32 + 1) * 32, :],
                                          raw[src:src + 32, :])
                t1 = rtmp.tile([128, QC], F32, name="t1", tag="t1")
                nc.vector.tensor_mul(t1[0:nr, :], raw[0:nr, :], ct[0:nr, :])
                nc.vector.tensor_mul(rot[0:nr, :], rot[0:nr, :], sn[0:nr, :])
                nc.vector.tensor_add(dst[0:nr, :], t1[0:nr, :], rot[0:nr, :])

            def qkv_block(b, sc):
                xt = xts[(b, sc)]
                for rt in range(2):
                    ps = gps.tile([128, QC], F32, name="gp", tag="gp")
                    for kd in range(NKD):
                        nc.tensor.matmul(
                            ps[:], wq_sb[kd // 4][:, kd % 4, rt * 128:(rt + 1) * 128],
                            xt[kd // 4][:, kd % 4, :],
                            start=(kd == 0), stop=(kd == NKD - 1))
                    rope(ps, sc, qT[rt][b][sc], 128)
                # packed [k(64, perm'd); v(64, natural)] tile
                ps = gps.tile([128, QC], F32, name="gp", tag="gp")
                for kd in range(NKD):
                    nc.tensor.matmul(ps[:], wkv_sb[:, kd, :],
                                     xt[kd // 4][:, kd % 4, :],
                                     start=(kd == 0), stop=(kd == NKD - 1))
                # k rows 0:64 -> rope -> duplicate into both kdup halves
                kr = rtmp.tile([64, QC], BF, name="kr", tag="kr")
                rope(ps, sc, kr, 64)
                nc.gpsimd.tensor_copy(kdup[b][sc][0:64, :], kr[:])
                nc.gpsimd.tensor_copy(kdup[b][sc][64:128, :], kr[:])
                # v rows 64:128 -> bf16 -> DMA-transpose into token-major vaug
                vtmp = vtp.tile([64, QC], BF, name="vtmp", tag="vtmp")
                nc.scalar.copy(vtmp[:], ps[64:128, :])
                for tt in range(4):
                    kt = sc * 4 + tt
                    nc.sync.dma_start_transpose(
                        vaug[b][kt][:, 0:64], vtmp[:, tt * 128:(tt + 1) * 128])

            def attn_phase(i):
                qc, q0, qn = SUBS[i]
                gqs = qc * QC + q0          # 128-aligned global q start
                t0 = gqs // 128             # first diagonal key tile
                nkt = (gqs + qn) // 128     # causal: key tiles up to sub end
                for b in range(NB):
                    atile = apool.tile([128, 2, QC], BF, name="atile", tag="atile")
                    for rt in range(2):
                        avs = [avps.tile([65, QC], F32, name="av", tag="av")
                               for _ in range(2)]
                        for kt in range(nkt):
                            kb = (kt % 4) * 128
                            m = kt - t0
                            qo = 128 * m if m > 0 else 0
                            n = qn - qo
                            st = stps.tile([128, 2, QC], F32, name="st", tag="st")
                            for half in range(2):
                                lo, hi = half * 64, half * 64 + 64
                                nc.tensor.matmul(
                                    st[:, half, 0:n],
                                    kdup[b][kt // 4][lo:hi, kb:kb + 128],
                                    qT[rt][b][qc][lo:hi, q0 + qo:q0 + qn],
                                    start=True, stop=(m < 0))
                                if m >= 0:  # fold causal mask into the PSUM
                                    nc.tensor.matmul(
                                        st[:, half, 0:128], mmc_sb[:, 0, :],
                                        mmc_sb[:, 1, :], start=False, stop=True)
                            p = ppool.tile([128, 2, QC], BF, name="p", tag="p")
                            nc.scalar.activation(p[:, :, 0:n], st[:, :, 0:n],
                                                 EXP, scale=SCALE)
                            for half in range(2):
                                nc.tensor.matmul(avs[half][:, qo:qn],
                                                 vaug[b][kt][:, :],
                                                 p[:, half, 0:n],
                                                 start=(kt == 0),
                                                 stop=(kt == nkt - 1))
                        for half in range(2):
                            av = avs[half]
                            recip = npool.tile([1, QC], F32, name="recip",
                                               tag="recip")
                            nc.vector.reciprocal(recip[:, 0:qn], av[64:65, 0:qn])
                            rb = npool.tile([64, QC], F32, name="rb", tag="rb")
                            nc.gpsimd.partition_broadcast(rb[:, 0:qn],
                                                          recip[:, 0:qn])
                            nc.vector.tensor_mul(
                                atile[half * 64:(half + 1) * 64, rt, 0:qn],
                                av[0:64, 0:qn], rb[:, 0:qn])
                    # stage to DRAM (ACT queue): dest 4b+d gets token block d
                    for r in range(2):
                        nc.scalar.dma_start(
                            a2a_in[i][4 * b:4 * b + 4, r, :, :]
                            .rearrange("d p t -> p d t"),
                            atile[:, r, 0:qn].rearrange("p (d t) -> p d t",
                                                        d=4))
                nc.gpsimd.collective_compute(
                    "AllToAll", mybir.AluOpType.bypass,
                    replica_groups=[[0, 1, 2, 3, 4, 5, 6, 7]],
                    ins=[a2a_in[i].opt()], outs=[a2a_out[i].opt()])

            agts = {}

            def gather(i, agt=None, col0=0):
                t4 = SUB_T4[i]
                if agt is None:
                    agt = agp.tile([128, NKD, 128], BF, name="agt", tag="agt")
                nc.sync.dma_start(
                    agt[:, :, col0:col0 + t4],
                    a2a_out[i].rearrange("c r p t -> p (c r) t"))
                agts[i] = agt
                return agt

            def wo_chain(i, ntok):
                """token-major wo for sub i's gathered tokens (<=128)."""
                agt = agts[i]
                for oc in range(4):
                    ps = gps.tile([128, QC], F32, name="gp", tag="gp")
                    for kd in range(NKD):
                        nc.tensor.matmul(
                            ps[0:ntok, :], agt[:, kd, 0:ntok],
                            woT_sb[:, kd, oc * QC:(oc + 1) * QC],
                            start=(kd == 0), stop=(kd == NKD - 1))
                    ot = opool.tile([128, QC], F32, name="ot", tag="ot")
                    nc.vector.tensor_copy(ot[0:ntok, :], ps[0:ntok, :])
                    nc.scalar.dma_start(
                        out[SUB_LOCAL[i]:SUB_LOCAL[i] + ntok,
                            oc * QC:(oc + 1) * QC],
                        ot[0:ntok, :])

            # ---- schedule ----
            # i=0: qkv(0) attn(0) | i=1: qkv(1) attn(1) | i=2: qkv(2) attn(2)
            # wo(0) | i=3: qkv(3) attn(3) wo(1) wo(2) | i=4: attn(4) wo(3+4)
            for i, (qc, q0, qn) in enumerate(SUBS):
                if q0 == 0:
                    qkv_block(0, qc)
                    consume_pending(1)
                    qkv_block(1, qc)
                    consume_pending(3)
                attn_phase(i)
                if i == 2:
                    gather(0)
                    wo_chain(0, 128)
                if i == 3:
                    gather(1)
                    wo_chain(1, 128)
                    gather(2)
                    wo_chain(2, 128)
            # tail: wo(3) fills the a2a(4) window (keeps the PE clock warm),
            # wo(4) is the only post-collective work
            gather(3)
            wo_chain(3, 96)
            gather(4)
            wo_chain(4, 32)

    nc.compile()
    return nc


def _get_nc():
    global _NC
    if _NC is None:
        _NC = _build()
    return _NC


def _prepare_in_maps(x, freqs_cis, wqkv, wo):
    x = np.asarray(x)
    freqs_cis = np.asarray(freqs_cis)
    wqkv = np.asarray(wqkv)
    wo = np.asarray(wo)

    perm = np.concatenate([np.arange(0, HD, 2), np.arange(1, HD, 2)])
    cos = np.ascontiguousarray(freqs_cis[:, :, 0].T)  # (32, S)
    sin = np.ascontiguousarray(freqs_cis[:, :, 1].T)
    cosS = np.ascontiguousarray(np.concatenate([cos, cos, cos, cos], axis=0),
                                dtype=np.float32)
    sinS = np.ascontiguousarray(np.concatenate([-sin, sin, -sin, sin], axis=0),
                                dtype=np.float32)
    p_i = np.arange(128)[:, None]
    f_i = np.arange(128)[None, :]
    mmcA = np.stack([-30000.0 * (f_i == p_i), 1.0 * (f_i < p_i)],
                    axis=1).astype(BF16)

    def stage(wt):
        # (D, C) with D = 16*128 -> (128, 16, C), per-partition contiguous
        return np.ascontiguousarray(
            wt.reshape(NKD, 128, wt.shape[1]).transpose(1, 0, 2)).astype(BF16)

    xSs = np.empty((NB, NSC, 128, NKD, QC), dtype=BF16)
    for b in range(NB):
        xt = x[b].T  # (D, S)
        xSs[b] = xt.reshape(NKD, 128, NSC, QC).transpose(2, 1, 0, 3)
    xSs = np.ascontiguousarray(xSs)

    # wo.T staged by attention-column order: kd=(src_core, rt), p=half*64+d
    # -> attn col (4*src + 2*rt + half)*64 + d ; identical for every core.
    p_idx = np.arange(128)
    kd_idx = np.arange(NKD)
    cols = ((4 * (kd_idx[None, :] // 2) + 2 * (kd_idx[None, :] % 2)
             + (p_idx[:, None] // 64)) * 64 + (p_idx[:, None] % 64))
    woTS = np.ascontiguousarray(
        wo[:, cols].transpose(1, 2, 0)).astype(BF16)  # [128, 16, 2048]

    in_maps = []
    for c in range(8):
        qrows = np.concatenate([(4 * c + h) * HD + perm for h in range(4)])
        krows = D + c * HD + perm
        vrows = D + 512 + c * HD + np.arange(HD)
        kvrows = np.concatenate([krows, vrows])
        in_maps.append({
            "xS": xSs,
            "wqS": stage(wqkv[qrows, :].T),
            "wkvS": stage(wqkv[kvrows, :].T),
            "woTS": woTS,
            "cosS": cosS,
            "sinS": sinS,
            "mmc": mmcA,
        })
    return in_maps


def kernel(x, freqs_cis, wqkv, wo, _trace=False):
    in_maps = _prepare_in_maps(x, freqs_cis, wqkv, wo)
    res = run_bass_kernel_spmd(_get_nc(), in_maps, core_ids=list(range(8)),
                               trace=_trace)

    outf = np.empty((2, S, D), np.float32)
    for c in range(8):
        b, blk = c // 4, c % 4
        o = res.results[c]["out"]  # [512, 2048] fp32
        for i in range(len(SUBS)):
            t4 = SUB_T4[i]
            g0 = SUB_BASE[i] + blk * t4
            l0 = SUB_LOCAL[i]
            outf[b, g0:g0 + t4, :] = o[l0:l0 + t4, :]
    if _trace:
        kernel.last_exec_time_ns = res.exec_time_ns
        kernel.last_results = res
    return outf
